# revision 13
# baseline (speedup 1.0000x reference)
"""Trainium2 Bass kernel: 8-expert top-2 MoE layer (SwiGLU experts).

Sharding: expert parallelism across 8 NeuronCores. The host performs the
router (exact fp64 softmax/top-2, shipped as per-token combine weights) and
the all-to-all token dispatch as part of input sharding; the combine
scatter-add happens in output unsharding. The expert FFN (gate/up/down
matmuls, SwiGLU) runs on device in bf16 with fp32 PSUM accumulation.

Self-contained: hardcodes all shapes from the problem spec.
"""

import os

import numpy as np

# Problem constants
H = 1024  # hidden dim
I = 4096  # intermediate dim
E = 8  # experts
P = 128  # SBUF partitions

# Tiling constants
TB = 512  # tokens per block (matmul moving free dim)
IS = 1024  # intermediate features resident per weight chunk
N_SUPER = I // IS
IT = IS // P  # i-tiles per super chunk
HO = H // P  # h chunks (contraction tiles)
HH = H // 512  # output column halves for the down projection
NQ = IT // 2  # quarter sub-tiles for the super-0 weight load


def _blocks(Tc):
    """Token blocks, largest first: super 0's first block consumes the
    just-arriving weights at the slowest rate, and the smallest block
    lands last so the end-of-kernel flush is minimal."""
    assert Tc % P == 0 and Tc >= 256
    sizes = []
    rem = Tc
    while rem > 767:
        sizes.append(TB)
        rem -= TB
    if rem > 512:
        sizes.extend([rem - 256, 256])
    elif rem:
        sizes.append(rem)
    sizes.sort(reverse=True)
    if len(sizes) >= 3:
        sizes.insert(0, sizes.pop(-2))
    blocks = []
    t = 0
    for tb in sizes:
        blocks.append((t, tb))
        t += tb
    return blocks


def build_moe(Tc: int):
    """Build the per-core Bass program for Tc tokens (Tc % 128 == 0)."""
    import concourse.bass as bass  # noqa: F401
    import concourse.mybir as mybir
    import concourse.tile as tile
    from concourse import bacc

    blocks = _blocks(Tc)
    NW = Tc // P  # combine-weight columns
    last_t0, last_tb = blocks[-1]
    f32 = mybir.dt.float32
    bf16 = mybir.dt.bfloat16
    Alu = mybir.AluOpType
    Act = mybir.ActivationFunctionType

    nc = bacc.Bacc(
        "TRN2", target_bir_lowering=False, debug=False, num_devices=8
    )

    xT = nc.dram_tensor("xT", [H, Tc], bf16, kind="ExternalInput").ap()
    wg = nc.dram_tensor("wg", [H, I], bf16, kind="ExternalInput").ap()
    wu = nc.dram_tensor("wu", [H, I], bf16, kind="ExternalInput").ap()
    wd = nc.dram_tensor("wd", [I, H], bf16, kind="ExternalInput").ap()
    wal = nc.dram_tensor("wal", [P, NW], f32, kind="ExternalInput").ap()
    out = nc.dram_tensor("out", [Tc, H], f32, kind="ExternalOutput").ap()
    # Last super x last block bypasses the read-modify-write accumulate so
    # the kernel tail drains plain writes; the host adds it back in.
    out2 = nc.dram_tensor("out2", [last_tb, H], bf16, kind="ExternalOutput").ap()

    # Partition-major views: h (or i) split as outer*P + partition
    xT_r = xT.rearrange("(ho p) t -> p ho t", p=P)  # [128, 8, Tc]
    wg_r = wg.rearrange("(ho p) i -> p ho i", p=P)  # [128, 8, 4096]
    wu_r = wu.rearrange("(ho p) i -> p ho i", p=P)
    wd_r = wd.rearrange("(io p) h -> p io h", p=P)  # [128, 32, 1024]

    with tile.TileContext(nc) as tc:
        with (
            tc.tile_pool(name="singles", bufs=1) as singles,
            tc.tile_pool(name="xres", bufs=1) as xres,
            tc.tile_pool(name="w0", bufs=1) as w0pool,
            tc.tile_pool(name="weights", bufs=2) as wpool,
            tc.tile_pool(name="hp", bufs=2) as hpool,
            tc.tile_pool(name="ep", bufs=3) as epool,
            tc.tile_pool(name="pgu", bufs=2, space="PSUM") as pgu,
            tc.tile_pool(name="pout", bufs=3, space="PSUM") as pout,
        ):
            # Per-token renormalized top-2 combine weight (host-computed);
            # first needed at the first down-group eviction (~48 us), so its
            # DMA is deferred behind the critical prologue loads.
            wal_sb = singles.tile([P, NW], f32)

            # x stays resident all kernel: one tile per block. Block 0
            # loads first on the gpsimd queue; the rest stream on the
            # vector queue, both otherwise idle during the prologue.
            x_sb = []
            for bi, (t0, tb) in enumerate(blocks):
                x_sb.append(xres.tile([P, HO, tb], bf16, tag=f"x{bi}", name=f"x{bi}"))
            t0_0, tb_0 = blocks[0]
            nc.gpsimd.dma_start(x_sb[0], xT_r[:, :, t0_0 : t0_0 + tb_0])

            for sup in range(N_SUPER):
                i0 = sup * IS
                if sup == 0:
                    # Super 0's gate/up weights race the PE. Measured
                    # queue rates: SWDGE (gpsimd) sustains ~260 GB/s while
                    # each HWDGE queue (sync/scalar) gives only ~60 GB/s,
                    # so the critical stream rides gpsimd in need-order
                    # (x0, then quarter pairs 1+3) with quarter pairs 0+2
                    # on sync/scalar.
                    wge, wue, wgq, wuq = [], [], [None], [None]
                    for s in range(2):
                        wge.append(
                            w0pool.tile([P, HO, P], bf16, tag=f"wge{s}", name=f"wge{s}")
                        )
                        wue.append(
                            w0pool.tile([P, HO, P], bf16, tag=f"wue{s}", name=f"wue{s}")
                        )
                    for q in range(1, NQ):
                        wgq.append(
                            w0pool.tile([P, HO, 2 * P], bf16, tag=f"wgq{q}", name=f"wgq{q}")
                        )
                        wuq.append(
                            w0pool.tile([P, HO, 2 * P], bf16, tag=f"wuq{q}", name=f"wuq{q}")
                        )

                    def _wslice(q):
                        c = i0 + q * 2 * P
                        return wg_r[:, :, c : c + 2 * P], wu_r[:, :, c : c + 2 * P]

                    for s in range(2):
                        c = i0 + s * P
                        nc.sync.dma_start(wge[s], wg_r[:, :, c : c + P])
                        nc.scalar.dma_start(wue[s], wu_r[:, :, c : c + P])
                    g1, u1 = _wslice(1)
                    nc.gpsimd.dma_start(wgq[1], g1)
                    nc.gpsimd.dma_start(wuq[1], u1)
                    g2, u2 = _wslice(2)
                    nc.sync.dma_start(wgq[2], g2)
                    nc.scalar.dma_start(wuq[2], u2)
                    g3, u3 = _wslice(3)
                    nc.gpsimd.dma_start(wgq[3], g3)
                    nc.gpsimd.dma_start(wuq[3], u3)

                    def wgt(it, wge=wge, wgq=wgq):
                        if it < 2:
                            return wge[it]
                        return wgq[it // 2][:, :, (it % 2) * P : (it % 2 + 1) * P]

                    def wut(it, wue=wue, wuq=wuq):
                        if it < 2:
                            return wue[it]
                        return wuq[it // 2][:, :, (it % 2) * P : (it % 2 + 1) * P]

                    nc.gpsimd.dma_start(x_sb[1], xT_r[:, :, blocks[1][0] : blocks[1][0] + blocks[1][1]])
                    nc.gpsimd.dma_start(wal_sb, wal)
                    wd_sb = wpool.tile([P, IT, H], bf16, tag="wd", name="wd")
                    nc.gpsimd.dma_start(
                        wd_sb, wd_r[:, sup * IT : (sup + 1) * IT, :]
                    )
                    for bi, (t0, tb) in enumerate(blocks):
                        if bi > 1:
                            nc.gpsimd.dma_start(
                                x_sb[bi], xT_r[:, :, t0 : t0 + tb]
                            )
                else:
                    wg_sb = wpool.tile([P, HO, IS], bf16, tag="wg", name="wg_sb")
                    nc.sync.dma_start(wg_sb, wg_r[:, :, i0 : i0 + IS])
                    wu_sb = wpool.tile([P, HO, IS], bf16, tag="wu", name="wu_sb")
                    nc.scalar.dma_start(wu_sb, wu_r[:, :, i0 : i0 + IS])

                    def wgt(it, wg_sb=wg_sb):
                        return wg_sb[:, :, it * P : (it + 1) * P]

                    def wut(it, wu_sb=wu_sb):
                        return wu_sb[:, :, it * P : (it + 1) * P]

                    wd_sb = wpool.tile([P, IT, H], bf16, tag="wd", name="wd")
                    nc.sync.dma_start(
                        wd_sb, wd_r[:, sup * IT : (sup + 1) * IT, :]
                    )

                def down_group(t0, h_sb, grp, sup=sup, wd_sb=wd_sb):
                    # One (token-subtile, output-half) group of the down
                    # projection, back to token-partition layout, scaled by
                    # the combine weight at PSUM eviction; partial sums over
                    # i-chunks accumulate directly in DRAM. Emitted
                    # interleaved with the next block's h production so the
                    # DVE evictions keep PSUM slots recycling.
                    tsub, hh = divmod(grp, HH)
                    col = t0 // P + tsub
                    r0 = t0 + tsub * P
                    ops = pout.tile([P, 512], f32, tag="o", name="o")
                    for it in range(IT):
                        nc.tensor.matmul(
                            ops,
                            lhsT=h_sb[:, it, tsub * P : (tsub + 1) * P],
                            rhs=wd_sb[:, it, hh * 512 : (hh + 1) * 512],
                            start=(it == 0),
                            stop=(it == IT - 1),
                        )
                    if sup == N_SUPER - 1 and t0 == last_t0:
                        oev2 = epool.tile([P, 512], bf16, tag="oev2", name="ov2")
                        nc.vector.tensor_scalar_mul(
                            oev2, ops, wal_sb[:, col : col + 1]
                        )
                        nc.gpsimd.dma_start(
                            out2[r0 - last_t0 : r0 - last_t0 + P,
                                 hh * 512 : (hh + 1) * 512],
                            oev2,
                        )
                    else:
                        oev = epool.tile([P, 512], f32, tag="oev", name="oev")
                        nc.vector.tensor_scalar_mul(
                            oev, ops, wal_sb[:, col : col + 1]
                        )
                        nc.gpsimd.dma_start(
                            out[r0 : r0 + P, hh * 512 : (hh + 1) * 512],
                            oev,
                            accum_op=(Alu.bypass if sup == 0 else Alu.add),
                        )

                pending = None
                for bi, (t0, tb) in enumerate(blocks):
                    tsn = tb // P
                    # Expert FFN for this (i-chunk, token block):
                    # hT[i, t] = silu(Wg.T x)[i, t] * (Wu.T x)[i, t]
                    h_sb = hpool.tile([P, IT, TB], bf16, tag="h", name="h")[:, :, :tb]
                    dgn = tsn * HH
                    for it in range(IT):
                        gps = pgu.tile([P, TB], f32, tag="g", name="g")[:, :tb]
                        ups = pgu.tile([P, TB], f32, tag="u", name="u")[:, :tb]
                        for ho in range(HO):
                            nc.tensor.matmul(
                                gps,
                                lhsT=wgt(it)[:, ho, :],
                                rhs=x_sb[bi][:, ho, :],
                                start=(ho == 0),
                                stop=(ho == HO - 1),
                            )
                        for ho in range(HO):
                            nc.tensor.matmul(
                                ups,
                                lhsT=wut(it)[:, ho, :],
                                rhs=x_sb[bi][:, ho, :],
                                start=(ho == 0),
                                stop=(ho == HO - 1),
                            )
                        gs = epool.tile([P, TB], f32, tag="gs", name="gs")[:, :tb]
                        nc.scalar.activation(gs, gps, Act.Silu)
                        nc.vector.tensor_tensor(
                            h_sb[:, it, :], gs, ups, op=Alu.mult
                        )
                        if pending is not None:
                            p_t0, p_h, p_dgn = pending
                            for grp in range(
                                it * p_dgn // IT, (it + 1) * p_dgn // IT
                            ):
                                down_group(p_t0, p_h, grp)

                    pending = (t0, h_sb, dgn)
                if pending is not None:
                    p_t0, p_h, p_dgn = pending
                    for grp in range(p_dgn):
                        down_group(p_t0, p_h, grp)

    nc.compile()
    return nc


def _run_spmd(nc, in_maps, trace):
    from concourse import bass_utils

    if trace:
        try:
            res = bass_utils.run_bass_kernel_spmd(
                nc, in_maps, core_ids=list(range(E)), trace=True
            )
            if res.exec_time_ns is not None:
                print(f"HW exec time: {res.exec_time_ns} ns")
            return res
        except Exception as exc:  # fall back to an untraced run
            print(f"traced run failed ({exc!r}); retrying without trace")
    return bass_utils.run_bass_kernel_spmd(
        nc, in_maps, core_ids=list(range(E)), trace=False
    )


def prepare(hidden_states, gate_proj_w, gate_weights, up_weights, down_weights):
    """Host router + dispatch; returns (nc, in_maps, combine_fn)."""
    import ml_dtypes

    bf16 = ml_dtypes.bfloat16
    x = np.ascontiguousarray(hidden_states, dtype=np.float32)
    gpw = np.ascontiguousarray(gate_proj_w, dtype=np.float32)
    T = x.shape[0]

    # Router in fp64: logits -> softmax -> top-2 (stable ties like
    # jax.lax.top_k) -> renormalized combine weights.
    logits = x.astype(np.float64) @ gpw.astype(np.float64).T  # [T, E]
    pr = np.exp(logits - logits.max(axis=1, keepdims=True))
    pr /= pr.sum(axis=1, keepdims=True)
    top2 = np.argsort(-pr, axis=1, kind="stable")[:, :2]
    pv = np.take_along_axis(pr, top2, axis=1)
    wts = (pv / pv.sum(axis=1, keepdims=True)).astype(np.float32)  # [T, 2]

    idx = [np.nonzero((top2 == e).any(axis=1))[0] for e in range(E)]
    mx = max(len(ix) for ix in idx)
    Tc = max(256, ((mx + P - 1) // P) * P)
    NW = Tc // P
    last_t0, last_tb = _blocks(Tc)[-1]

    nc = build_moe(Tc)
    in_maps = []
    for e in range(E):
        n_e = len(idx[e])
        xTe = np.zeros((H, Tc), dtype=bf16)
        if n_e:
            xTe[:, :n_e] = np.ascontiguousarray(x[idx[e]].T).astype(bf16)
        we = np.zeros((Tc,), dtype=np.float32)
        if n_e:
            we[:n_e] = np.where(
                top2[idx[e], 0] == e, wts[idx[e], 0], wts[idx[e], 1]
            )
        in_maps.append(
            {
                "xT": xTe,
                "wg": np.ascontiguousarray(gate_weights[e]).astype(bf16),
                "wu": np.ascontiguousarray(up_weights[e]).astype(bf16),
                "wd": np.ascontiguousarray(down_weights[e]).astype(bf16),
                "wal": np.ascontiguousarray(we.reshape(NW, P).T),
            }
        )

    def combine(results):
        out = np.zeros((T, H), dtype=np.float32)
        for e in range(E):
            n_e = len(idx[e])
            if n_e:
                full = results[e]["out"].copy()
                full[last_t0 : last_t0 + last_tb] += results[e]["out2"].astype(np.float32)
                out[idx[e]] += full[:n_e]
        return out

    return nc, in_maps, combine


def kernel(hidden_states, gate_proj_w, gate_weights, up_weights, down_weights):
    trace = os.environ.get("MOE_TRACE", "0") == "1"
    nc, in_maps, combine = prepare(
        hidden_states, gate_proj_w, gate_weights, up_weights, down_weights
    )
    res = _run_spmd(nc, in_maps, trace)
    return combine(res.results)


# revision 15
# speedup vs baseline: 1.0260x; 1.0260x over previous
"""Trainium2 Bass kernel: 8-expert top-2 MoE layer (SwiGLU experts).

Sharding: expert parallelism across 8 NeuronCores. The host performs the
router (exact fp64 softmax/top-2, shipped as per-token combine weights) and
the all-to-all token dispatch as part of input sharding; the combine
scatter-add happens in output unsharding. The expert FFN (gate/up/down
matmuls, SwiGLU) runs on device in bf16 with fp32 PSUM accumulation.

Self-contained: hardcodes all shapes from the problem spec.
"""

import os

import numpy as np

# Problem constants
H = 1024  # hidden dim
I = 4096  # intermediate dim
E = 8  # experts
P = 128  # SBUF partitions

# Tiling constants
TB = 512  # tokens per block (matmul moving free dim)
IS = 1024  # intermediate features resident per weight chunk
N_SUPER = I // IS
IT = IS // P  # i-tiles per super chunk
HO = H // P  # h chunks (contraction tiles)
HH = H // 512  # output column halves for the down projection
NQ = IT // 2  # quarter sub-tiles for the super-0 weight load


def _blocks(Tc):
    """Token blocks, largest first: super 0's first block consumes the
    just-arriving weights at the slowest rate, and the smallest block
    lands last so the end-of-kernel flush is minimal."""
    assert Tc % P == 0 and Tc >= 256
    sizes = []
    rem = Tc
    while rem > 767:
        sizes.append(TB)
        rem -= TB
    if rem > 512:
        sizes.extend([rem - 256, 256])
    elif rem:
        sizes.append(rem)
    sizes.sort(reverse=True)
    blocks = []
    t = 0
    for tb in sizes:
        blocks.append((t, tb))
        t += tb
    return blocks


def build_moe(Tc: int):
    """Build the per-core Bass program for Tc tokens (Tc % 128 == 0)."""
    import concourse.bass as bass  # noqa: F401
    import concourse.mybir as mybir
    import concourse.tile as tile
    from concourse import bacc

    blocks = _blocks(Tc)
    NW = Tc // P  # combine-weight columns
    f32 = mybir.dt.float32
    bf16 = mybir.dt.bfloat16
    Alu = mybir.AluOpType
    Act = mybir.ActivationFunctionType

    nc = bacc.Bacc(
        "TRN2", target_bir_lowering=False, debug=False, num_devices=8
    )

    xT = nc.dram_tensor("xT", [H, Tc], bf16, kind="ExternalInput").ap()
    wg = nc.dram_tensor("wg", [H, I], bf16, kind="ExternalInput").ap()
    wu = nc.dram_tensor("wu", [H, I], bf16, kind="ExternalInput").ap()
    wd = nc.dram_tensor("wd", [I, H], bf16, kind="ExternalInput").ap()
    wal = nc.dram_tensor("wal", [P, NW], f32, kind="ExternalInput").ap()
    out = nc.dram_tensor("out", [Tc, H], f32, kind="ExternalOutput").ap()
    # The last super bypasses the read-modify-write accumulate entirely:
    # its partials land in a separate bf16 buffer as plain writes (so the
    # kernel tail drains fast) and the host adds them during combine.
    out2 = nc.dram_tensor("out2", [Tc, H], bf16, kind="ExternalOutput").ap()

    # Partition-major views: h (or i) split as outer*P + partition
    xT_r = xT.rearrange("(ho p) t -> p ho t", p=P)  # [128, 8, Tc]
    wg_r = wg.rearrange("(ho p) i -> p ho i", p=P)  # [128, 8, 4096]
    wu_r = wu.rearrange("(ho p) i -> p ho i", p=P)
    wd_r = wd.rearrange("(io p) h -> p io h", p=P)  # [128, 32, 1024]

    with tile.TileContext(nc) as tc:
        with (
            tc.tile_pool(name="singles", bufs=1) as singles,
            tc.tile_pool(name="xres", bufs=1) as xres,
            tc.tile_pool(name="w0", bufs=1) as w0pool,
            tc.tile_pool(name="weights", bufs=2) as wpool,
            tc.tile_pool(name="hp", bufs=2) as hpool,
            tc.tile_pool(name="ep", bufs=3) as epool,
            tc.tile_pool(name="pgu", bufs=2, space="PSUM") as pgu,
            tc.tile_pool(name="pout", bufs=3, space="PSUM") as pout,
        ):
            # Per-token renormalized top-2 combine weight (host-computed);
            # first needed at the first down-group eviction (~48 us), so its
            # DMA is deferred behind the critical prologue loads.
            wal_sb = singles.tile([P, NW], f32)

            # x stays resident all kernel: one tile per block. Block 0
            # loads first on the gpsimd queue; the rest stream on the
            # vector queue, both otherwise idle during the prologue.
            x_sb = []
            for bi, (t0, tb) in enumerate(blocks):
                x_sb.append(xres.tile([P, HO, tb], bf16, tag=f"x{bi}", name=f"x{bi}"))
            t0_0, tb_0 = blocks[0]
            nc.gpsimd.dma_start(x_sb[0], xT_r[:, :, t0_0 : t0_0 + tb_0])

            for sup in range(N_SUPER):
                i0 = sup * IS
                if sup == 0:
                    # Super 0's gate/up weights race the PE. Measured
                    # queue rates: SWDGE (gpsimd) sustains ~260 GB/s while
                    # each HWDGE queue (sync/scalar) gives only ~60 GB/s,
                    # so the critical stream rides gpsimd in need-order
                    # (x0, then quarter pairs 1+3) with quarter pairs 0+2
                    # on sync/scalar.
                    wgq, wuq = [], []
                    for q in range(NQ):
                        wgq.append(
                            w0pool.tile([P, HO, 2 * P], bf16, tag=f"wgq{q}", name=f"wgq{q}")
                        )
                        wuq.append(
                            w0pool.tile([P, HO, 2 * P], bf16, tag=f"wuq{q}", name=f"wuq{q}")
                        )

                    def _wslice(q):
                        c = i0 + q * 2 * P
                        return wg_r[:, :, c : c + 2 * P], wu_r[:, :, c : c + 2 * P]

                    g0, u0 = _wslice(0)
                    nc.sync.dma_start(wgq[0], g0)
                    nc.scalar.dma_start(wuq[0], u0)
                    g1, u1 = _wslice(1)
                    nc.gpsimd.dma_start(wgq[1], g1)
                    nc.gpsimd.dma_start(wuq[1], u1)
                    g2, u2 = _wslice(2)
                    nc.sync.dma_start(wgq[2], g2)
                    nc.scalar.dma_start(wuq[2], u2)
                    g3, u3 = _wslice(3)
                    nc.gpsimd.dma_start(wgq[3], g3)
                    nc.gpsimd.dma_start(wuq[3], u3)

                    def wgt(it, wgq=wgq):
                        return wgq[it // 2][:, :, (it % 2) * P : (it % 2 + 1) * P]

                    def wut(it, wuq=wuq):
                        return wuq[it // 2][:, :, (it % 2) * P : (it % 2 + 1) * P]

                    nc.gpsimd.dma_start(x_sb[1], xT_r[:, :, blocks[1][0] : blocks[1][0] + blocks[1][1]])
                    nc.gpsimd.dma_start(wal_sb, wal)
                    wd_sb = wpool.tile([P, IT, H], bf16, tag="wd", name="wd")
                    nc.gpsimd.dma_start(
                        wd_sb, wd_r[:, sup * IT : (sup + 1) * IT, :]
                    )
                    for bi, (t0, tb) in enumerate(blocks):
                        if bi > 1:
                            nc.gpsimd.dma_start(
                                x_sb[bi], xT_r[:, :, t0 : t0 + tb]
                            )
                else:
                    wg_sb = wpool.tile([P, HO, IS], bf16, tag="wg", name="wg_sb")
                    nc.sync.dma_start(wg_sb, wg_r[:, :, i0 : i0 + IS])
                    wu_sb = wpool.tile([P, HO, IS], bf16, tag="wu", name="wu_sb")
                    nc.scalar.dma_start(wu_sb, wu_r[:, :, i0 : i0 + IS])

                    def wgt(it, wg_sb=wg_sb):
                        return wg_sb[:, :, it * P : (it + 1) * P]

                    def wut(it, wu_sb=wu_sb):
                        return wu_sb[:, :, it * P : (it + 1) * P]

                    wd_sb = wpool.tile([P, IT, H], bf16, tag="wd", name="wd")
                    nc.sync.dma_start(
                        wd_sb, wd_r[:, sup * IT : (sup + 1) * IT, :]
                    )

                def down_group(t0, h_sb, grp, sup=sup, wd_sb=wd_sb):
                    # One (token-subtile, output-half) group of the down
                    # projection, back to token-partition layout, scaled by
                    # the combine weight at PSUM eviction; partial sums over
                    # i-chunks accumulate directly in DRAM. Emitted
                    # interleaved with the next block's h production so the
                    # DVE evictions keep PSUM slots recycling.
                    tsub, hh = divmod(grp, HH)
                    col = t0 // P + tsub
                    r0 = t0 + tsub * P
                    ops = pout.tile([P, 512], f32, tag="o", name="o")
                    for it in range(IT):
                        nc.tensor.matmul(
                            ops,
                            lhsT=h_sb[:, it, tsub * P : (tsub + 1) * P],
                            rhs=wd_sb[:, it, hh * 512 : (hh + 1) * 512],
                            start=(it == 0),
                            stop=(it == IT - 1),
                        )
                    if sup == N_SUPER - 1:
                        oev2 = epool.tile([P, 512], bf16, tag="oev2", name="ov2")
                        nc.vector.tensor_scalar_mul(
                            oev2, ops, wal_sb[:, col : col + 1]
                        )
                        nc.gpsimd.dma_start(
                            out2[r0 : r0 + P, hh * 512 : (hh + 1) * 512],
                            oev2,
                        )
                    else:
                        oev = epool.tile([P, 512], f32, tag="oev", name="oev")
                        nc.vector.tensor_scalar_mul(
                            oev, ops, wal_sb[:, col : col + 1]
                        )
                        nc.gpsimd.dma_start(
                            out[r0 : r0 + P, hh * 512 : (hh + 1) * 512],
                            oev,
                            accum_op=(Alu.bypass if sup == 0 else Alu.add),
                        )

                pending = None
                for bi, (t0, tb) in enumerate(blocks):
                    tsn = tb // P
                    # Expert FFN for this (i-chunk, token block):
                    # hT[i, t] = silu(Wg.T x)[i, t] * (Wu.T x)[i, t]
                    h_sb = hpool.tile([P, IT, TB], bf16, tag="h", name="h")[:, :, :tb]
                    dgn = tsn * HH
                    for it in range(IT):
                        gps = pgu.tile([P, TB], f32, tag="g", name="g")[:, :tb]
                        ups = pgu.tile([P, TB], f32, tag="u", name="u")[:, :tb]
                        for ho in range(HO):
                            nc.tensor.matmul(
                                gps,
                                lhsT=wgt(it)[:, ho, :],
                                rhs=x_sb[bi][:, ho, :],
                                start=(ho == 0),
                                stop=(ho == HO - 1),
                            )
                        for ho in range(HO):
                            nc.tensor.matmul(
                                ups,
                                lhsT=wut(it)[:, ho, :],
                                rhs=x_sb[bi][:, ho, :],
                                start=(ho == 0),
                                stop=(ho == HO - 1),
                            )
                        gs = epool.tile([P, TB], f32, tag="gs", name="gs")[:, :tb]
                        nc.scalar.activation(gs, gps, Act.Silu)
                        nc.vector.tensor_tensor(
                            h_sb[:, it, :], gs, ups, op=Alu.mult
                        )
                        if pending is not None:
                            p_t0, p_h, p_dgn = pending
                            for grp in range(
                                it * p_dgn // IT, (it + 1) * p_dgn // IT
                            ):
                                down_group(p_t0, p_h, grp)

                    pending = (t0, h_sb, dgn)
                if pending is not None:
                    p_t0, p_h, p_dgn = pending
                    for grp in range(p_dgn):
                        down_group(p_t0, p_h, grp)

    nc.compile()
    return nc


def _run_spmd(nc, in_maps, trace):
    from concourse import bass_utils

    if trace:
        try:
            res = bass_utils.run_bass_kernel_spmd(
                nc, in_maps, core_ids=list(range(E)), trace=True
            )
            if res.exec_time_ns is not None:
                print(f"HW exec time: {res.exec_time_ns} ns")
            return res
        except Exception as exc:  # fall back to an untraced run
            print(f"traced run failed ({exc!r}); retrying without trace")
    return bass_utils.run_bass_kernel_spmd(
        nc, in_maps, core_ids=list(range(E)), trace=False
    )


def prepare(hidden_states, gate_proj_w, gate_weights, up_weights, down_weights):
    """Host router + dispatch; returns (nc, in_maps, combine_fn)."""
    import ml_dtypes

    bf16 = ml_dtypes.bfloat16
    x = np.ascontiguousarray(hidden_states, dtype=np.float32)
    gpw = np.ascontiguousarray(gate_proj_w, dtype=np.float32)
    T = x.shape[0]

    # Router in fp64: logits -> softmax -> top-2 (stable ties like
    # jax.lax.top_k) -> renormalized combine weights.
    logits = x.astype(np.float64) @ gpw.astype(np.float64).T  # [T, E]
    pr = np.exp(logits - logits.max(axis=1, keepdims=True))
    pr /= pr.sum(axis=1, keepdims=True)
    top2 = np.argsort(-pr, axis=1, kind="stable")[:, :2]
    pv = np.take_along_axis(pr, top2, axis=1)
    wts = (pv / pv.sum(axis=1, keepdims=True)).astype(np.float32)  # [T, 2]

    idx = [np.nonzero((top2 == e).any(axis=1))[0] for e in range(E)]
    mx = max(len(ix) for ix in idx)
    Tc = max(256, ((mx + P - 1) // P) * P)
    NW = Tc // P

    nc = build_moe(Tc)
    in_maps = []
    for e in range(E):
        n_e = len(idx[e])
        xTe = np.zeros((H, Tc), dtype=bf16)
        if n_e:
            xTe[:, :n_e] = np.ascontiguousarray(x[idx[e]].T).astype(bf16)
        we = np.zeros((Tc,), dtype=np.float32)
        if n_e:
            we[:n_e] = np.where(
                top2[idx[e], 0] == e, wts[idx[e], 0], wts[idx[e], 1]
            )
        in_maps.append(
            {
                "xT": xTe,
                "wg": np.ascontiguousarray(gate_weights[e]).astype(bf16),
                "wu": np.ascontiguousarray(up_weights[e]).astype(bf16),
                "wd": np.ascontiguousarray(down_weights[e]).astype(bf16),
                "wal": np.ascontiguousarray(we.reshape(NW, P).T),
            }
        )

    def combine(results):
        out = np.zeros((T, H), dtype=np.float32)
        for e in range(E):
            n_e = len(idx[e])
            if n_e:
                full = results[e]["out"].copy()
                full += results[e]["out2"].astype(np.float32)
                out[idx[e]] += full[:n_e]
        return out

    return nc, in_maps, combine


def kernel(hidden_states, gate_proj_w, gate_weights, up_weights, down_weights):
    trace = os.environ.get("MOE_TRACE", "0") == "1"
    nc, in_maps, combine = prepare(
        hidden_states, gate_proj_w, gate_weights, up_weights, down_weights
    )
    res = _run_spmd(nc, in_maps, trace)
    return combine(res.results)


# revision 16
# speedup vs baseline: 1.0335x; 1.0073x over previous
"""Trainium2 Bass kernel: 8-expert top-2 MoE layer (SwiGLU experts).

Sharding: expert parallelism across 8 NeuronCores. The host performs the
router (exact fp64 softmax/top-2, shipped as per-token combine weights) and
the all-to-all token dispatch as part of input sharding; the combine
scatter-add happens in output unsharding. The expert FFN (gate/up/down
matmuls, SwiGLU) runs on device in bf16 with fp32 PSUM accumulation.

Self-contained: hardcodes all shapes from the problem spec.
"""

import os

import numpy as np

# Problem constants
H = 1024  # hidden dim
I = 4096  # intermediate dim
E = 8  # experts
P = 128  # SBUF partitions

# Tiling constants
TB = 512  # tokens per block (matmul moving free dim)
IS = 1024  # intermediate features resident per weight chunk
N_SUPER = I // IS
IT = IS // P  # i-tiles per super chunk
HO = H // P  # h chunks (contraction tiles)
HH = H // 512  # output column halves for the down projection
NQ = IT // 2  # quarter sub-tiles for the super-0 weight load


def _blocks(Tc):
    """Token blocks, largest first: super 0's first block consumes the
    just-arriving weights at the slowest rate, and the smallest block
    lands last so the end-of-kernel flush is minimal."""
    assert Tc % P == 0 and Tc >= 256
    sizes = []
    rem = Tc
    while rem > 767:
        sizes.append(TB)
        rem -= TB
    if rem > 512:
        sizes.extend([rem - 256, 256])
    elif rem:
        sizes.append(rem)
    sizes.sort(reverse=True)
    blocks = []
    t = 0
    for tb in sizes:
        blocks.append((t, tb))
        t += tb
    return blocks


def build_moe(Tc: int):
    """Build the per-core Bass program for Tc tokens (Tc % 128 == 0)."""
    import concourse.bass as bass  # noqa: F401
    import concourse.mybir as mybir
    import concourse.tile as tile
    from concourse import bacc

    blocks = _blocks(Tc)
    NW = Tc // P  # combine-weight columns
    f32 = mybir.dt.float32
    bf16 = mybir.dt.bfloat16
    Alu = mybir.AluOpType
    Act = mybir.ActivationFunctionType

    nc = bacc.Bacc(
        "TRN2", target_bir_lowering=False, debug=False, num_devices=8
    )

    xT = nc.dram_tensor("xT", [H, Tc], bf16, kind="ExternalInput").ap()
    wg = nc.dram_tensor("wg", [H, I], bf16, kind="ExternalInput").ap()
    wu = nc.dram_tensor("wu", [H, I], bf16, kind="ExternalInput").ap()
    wd = nc.dram_tensor("wd", [I, H], bf16, kind="ExternalInput").ap()
    wal = nc.dram_tensor("wal", [P, NW], f32, kind="ExternalInput").ap()
    out = nc.dram_tensor("out", [Tc, H], f32, kind="ExternalOutput").ap()
    # The last super bypasses the read-modify-write accumulate entirely:
    # its partials land in a separate bf16 buffer as plain writes (so the
    # kernel tail drains fast) and the host adds them during combine.
    out2 = nc.dram_tensor("out2", [Tc, H], bf16, kind="ExternalOutput").ap()

    # Partition-major views: h (or i) split as outer*P + partition
    xT_r = xT.rearrange("(ho p) t -> p ho t", p=P)  # [128, 8, Tc]
    wg_r = wg.rearrange("(ho p) i -> p ho i", p=P)  # [128, 8, 4096]
    wu_r = wu.rearrange("(ho p) i -> p ho i", p=P)
    wd_r = wd.rearrange("(io p) h -> p io h", p=P)  # [128, 32, 1024]

    with tile.TileContext(nc) as tc:
        with (
            tc.tile_pool(name="singles", bufs=1) as singles,
            tc.tile_pool(name="xres", bufs=1) as xres,
            tc.tile_pool(name="w0", bufs=1) as w0pool,
            tc.tile_pool(name="weights", bufs=2) as wpool,
            tc.tile_pool(name="hp", bufs=2) as hpool,
            tc.tile_pool(name="ep", bufs=3) as epool,
            tc.tile_pool(name="pgu", bufs=2, space="PSUM") as pgu,
            tc.tile_pool(name="pout", bufs=3, space="PSUM") as pout,
        ):
            # Per-token renormalized top-2 combine weight (host-computed);
            # first needed at the first down-group eviction (~48 us), so its
            # DMA is deferred behind the critical prologue loads.
            wal_sb = singles.tile([P, NW], f32)

            # x stays resident all kernel: one tile per block. Block 0
            # loads first on the gpsimd queue; the rest stream on the
            # vector queue, both otherwise idle during the prologue.
            x_sb = []
            for bi, (t0, tb) in enumerate(blocks):
                x_sb.append(xres.tile([P, HO, tb], bf16, tag=f"x{bi}", name=f"x{bi}"))
            t0_0, tb_0 = blocks[0]
            nc.gpsimd.dma_start(x_sb[0], xT_r[:, :, t0_0 : t0_0 + tb_0])

            for sup in range(N_SUPER):
                i0 = sup * IS
                if sup == 0:
                    # Super 0's gate/up weights race the PE. Measured
                    # queue rates: SWDGE (gpsimd) sustains ~260 GB/s while
                    # each HWDGE queue (sync/scalar) gives only ~60 GB/s,
                    # so the critical stream rides gpsimd in need-order
                    # (x0, then quarter pairs 1+3) with quarter pairs 0+2
                    # on sync/scalar.
                    wgq, wuq = [], []
                    for q in range(NQ):
                        wgq.append(
                            w0pool.tile([P, HO, 2 * P], bf16, tag=f"wgq{q}", name=f"wgq{q}")
                        )
                        wuq.append(
                            w0pool.tile([P, HO, 2 * P], bf16, tag=f"wuq{q}", name=f"wuq{q}")
                        )

                    def _wslice(q):
                        c = i0 + q * 2 * P
                        return wg_r[:, :, c : c + 2 * P], wu_r[:, :, c : c + 2 * P]

                    g0, u0 = _wslice(0)
                    nc.sync.dma_start(wgq[0], g0)
                    nc.scalar.dma_start(wuq[0], u0)
                    g1, u1 = _wslice(1)
                    nc.gpsimd.dma_start(wgq[1], g1)
                    nc.gpsimd.dma_start(wuq[1], u1)
                    g2, u2 = _wslice(2)
                    nc.sync.dma_start(wgq[2], g2)
                    nc.scalar.dma_start(wuq[2], u2)
                    g3, u3 = _wslice(3)
                    nc.gpsimd.dma_start(wgq[3], g3)
                    nc.gpsimd.dma_start(wuq[3], u3)

                    def wgt(it, wgq=wgq):
                        return wgq[it // 2][:, :, (it % 2) * P : (it % 2 + 1) * P]

                    def wut(it, wuq=wuq):
                        return wuq[it // 2][:, :, (it % 2) * P : (it % 2 + 1) * P]

                    nc.gpsimd.dma_start(x_sb[1], xT_r[:, :, blocks[1][0] : blocks[1][0] + blocks[1][1]])
                    nc.gpsimd.dma_start(wal_sb, wal)
                    wd_sb = wpool.tile([P, IT, H], bf16, tag="wd", name="wd")
                    nc.gpsimd.dma_start(
                        wd_sb, wd_r[:, sup * IT : (sup + 1) * IT, :]
                    )
                    for bi, (t0, tb) in enumerate(blocks):
                        if bi > 1:
                            nc.gpsimd.dma_start(
                                x_sb[bi], xT_r[:, :, t0 : t0 + tb]
                            )
                else:
                    wg_sb = wpool.tile([P, HO, IS], bf16, tag="wg", name="wg_sb")
                    nc.sync.dma_start(wg_sb, wg_r[:, :, i0 : i0 + IS])
                    wu_sb = wpool.tile([P, HO, IS], bf16, tag="wu", name="wu_sb")
                    nc.scalar.dma_start(wu_sb, wu_r[:, :, i0 : i0 + IS])

                    def wgt(it, wg_sb=wg_sb):
                        return wg_sb[:, :, it * P : (it + 1) * P]

                    def wut(it, wu_sb=wu_sb):
                        return wu_sb[:, :, it * P : (it + 1) * P]

                    wd_sb = wpool.tile([P, IT, H], bf16, tag="wd", name="wd")
                    nc.sync.dma_start(
                        wd_sb, wd_r[:, sup * IT : (sup + 1) * IT, :]
                    )

                def down_group(t0, h_sb, grp, sup=sup, wd_sb=wd_sb):
                    # One (token-subtile, output-half) group of the down
                    # projection, back to token-partition layout, scaled by
                    # the combine weight at PSUM eviction; partial sums over
                    # i-chunks accumulate directly in DRAM. Emitted
                    # interleaved with the next block's h production so the
                    # DVE evictions keep PSUM slots recycling.
                    tsub, hh = divmod(grp, HH)
                    col = t0 // P + tsub
                    r0 = t0 + tsub * P
                    ops = pout.tile([P, 512], f32, tag="o", name="o")
                    for it in range(IT):
                        nc.tensor.matmul(
                            ops,
                            lhsT=h_sb[:, it, tsub * P : (tsub + 1) * P],
                            rhs=wd_sb[:, it, hh * 512 : (hh + 1) * 512],
                            start=(it == 0),
                            stop=(it == IT - 1),
                        )
                    if sup == N_SUPER - 1:
                        # Plain writes to a disjoint tensor: ride the idle
                        # HWDGE queues so the SWDGE queue (and its ~7.5 us
                        # end-of-kernel drain) is long quiet by the tail.
                        oev2 = epool.tile([P, 512], bf16, tag="oev2", name="ov2")
                        nc.vector.tensor_scalar_mul(
                            oev2, ops, wal_sb[:, col : col + 1]
                        )
                        eng = nc.sync if (col + hh) % 2 == 0 else nc.scalar
                        eng.dma_start(
                            out2[r0 : r0 + P, hh * 512 : (hh + 1) * 512],
                            oev2,
                        )
                    else:
                        oev = epool.tile([P, 512], f32, tag="oev", name="oev")
                        nc.vector.tensor_scalar_mul(
                            oev, ops, wal_sb[:, col : col + 1]
                        )
                        nc.gpsimd.dma_start(
                            out[r0 : r0 + P, hh * 512 : (hh + 1) * 512],
                            oev,
                            accum_op=(Alu.bypass if sup == 0 else Alu.add),
                        )

                pending = None
                for bi, (t0, tb) in enumerate(blocks):
                    tsn = tb // P
                    # Expert FFN for this (i-chunk, token block):
                    # hT[i, t] = silu(Wg.T x)[i, t] * (Wu.T x)[i, t]
                    h_sb = hpool.tile([P, IT, TB], bf16, tag="h", name="h")[:, :, :tb]
                    dgn = tsn * HH
                    for it in range(IT):
                        gps = pgu.tile([P, TB], f32, tag="g", name="g")[:, :tb]
                        ups = pgu.tile([P, TB], f32, tag="u", name="u")[:, :tb]
                        for ho in range(HO):
                            nc.tensor.matmul(
                                gps,
                                lhsT=wgt(it)[:, ho, :],
                                rhs=x_sb[bi][:, ho, :],
                                start=(ho == 0),
                                stop=(ho == HO - 1),
                            )
                        for ho in range(HO):
                            nc.tensor.matmul(
                                ups,
                                lhsT=wut(it)[:, ho, :],
                                rhs=x_sb[bi][:, ho, :],
                                start=(ho == 0),
                                stop=(ho == HO - 1),
                            )
                        gs = epool.tile([P, TB], f32, tag="gs", name="gs")[:, :tb]
                        nc.scalar.activation(gs, gps, Act.Silu)
                        nc.vector.tensor_tensor(
                            h_sb[:, it, :], gs, ups, op=Alu.mult
                        )
                        if pending is not None:
                            p_t0, p_h, p_dgn = pending
                            for grp in range(
                                it * p_dgn // IT, (it + 1) * p_dgn // IT
                            ):
                                down_group(p_t0, p_h, grp)

                    pending = (t0, h_sb, dgn)
                if pending is not None:
                    p_t0, p_h, p_dgn = pending
                    for grp in range(p_dgn):
                        down_group(p_t0, p_h, grp)

    nc.compile()
    return nc


def _run_spmd(nc, in_maps, trace):
    from concourse import bass_utils

    if trace:
        try:
            res = bass_utils.run_bass_kernel_spmd(
                nc, in_maps, core_ids=list(range(E)), trace=True
            )
            if res.exec_time_ns is not None:
                print(f"HW exec time: {res.exec_time_ns} ns")
            return res
        except Exception as exc:  # fall back to an untraced run
            print(f"traced run failed ({exc!r}); retrying without trace")
    return bass_utils.run_bass_kernel_spmd(
        nc, in_maps, core_ids=list(range(E)), trace=False
    )


def prepare(hidden_states, gate_proj_w, gate_weights, up_weights, down_weights):
    """Host router + dispatch; returns (nc, in_maps, combine_fn)."""
    import ml_dtypes

    bf16 = ml_dtypes.bfloat16
    x = np.ascontiguousarray(hidden_states, dtype=np.float32)
    gpw = np.ascontiguousarray(gate_proj_w, dtype=np.float32)
    T = x.shape[0]

    # Router in fp64: logits -> softmax -> top-2 (stable ties like
    # jax.lax.top_k) -> renormalized combine weights.
    logits = x.astype(np.float64) @ gpw.astype(np.float64).T  # [T, E]
    pr = np.exp(logits - logits.max(axis=1, keepdims=True))
    pr /= pr.sum(axis=1, keepdims=True)
    top2 = np.argsort(-pr, axis=1, kind="stable")[:, :2]
    pv = np.take_along_axis(pr, top2, axis=1)
    wts = (pv / pv.sum(axis=1, keepdims=True)).astype(np.float32)  # [T, 2]

    idx = [np.nonzero((top2 == e).any(axis=1))[0] for e in range(E)]
    mx = max(len(ix) for ix in idx)
    Tc = max(256, ((mx + P - 1) // P) * P)
    NW = Tc // P

    nc = build_moe(Tc)
    in_maps = []
    for e in range(E):
        n_e = len(idx[e])
        xTe = np.zeros((H, Tc), dtype=bf16)
        if n_e:
            xTe[:, :n_e] = np.ascontiguousarray(x[idx[e]].T).astype(bf16)
        we = np.zeros((Tc,), dtype=np.float32)
        if n_e:
            we[:n_e] = np.where(
                top2[idx[e], 0] == e, wts[idx[e], 0], wts[idx[e], 1]
            )
        in_maps.append(
            {
                "xT": xTe,
                "wg": np.ascontiguousarray(gate_weights[e]).astype(bf16),
                "wu": np.ascontiguousarray(up_weights[e]).astype(bf16),
                "wd": np.ascontiguousarray(down_weights[e]).astype(bf16),
                "wal": np.ascontiguousarray(we.reshape(NW, P).T),
            }
        )

    def combine(results):
        out = np.zeros((T, H), dtype=np.float32)
        for e in range(E):
            n_e = len(idx[e])
            if n_e:
                full = results[e]["out"].copy()
                full += results[e]["out2"].astype(np.float32)
                out[idx[e]] += full[:n_e]
        return out

    return nc, in_maps, combine


def kernel(hidden_states, gate_proj_w, gate_weights, up_weights, down_weights):
    trace = os.environ.get("MOE_TRACE", "0") == "1"
    nc, in_maps, combine = prepare(
        hidden_states, gate_proj_w, gate_weights, up_weights, down_weights
    )
    res = _run_spmd(nc, in_maps, trace)
    return combine(res.results)


# revision 18
# speedup vs baseline: 1.0463x; 1.0124x over previous
"""Trainium2 Bass kernel: 8-expert top-2 MoE layer (SwiGLU experts).

Sharding: paired expert parallelism across 8 NeuronCores. Experts are
paired heaviest-with-lightest; each pair lands on two cores that both
process BOTH experts' full token sets over HALF of the intermediate dim
(an exact decomposition: gate/up split along their output dim, the down
projection's partial contractions summed on the host). Per-core work is
(max heavy load + max light load)/2 token-equivalents instead of the max
expert load. The host performs the router (exact fp64 softmax/top-2,
shipped as per-token combine weights) and the token dispatch/combine.
The FFN runs in bf16 with fp32 PSUM accumulation.

Self-contained: hardcodes all shapes from the problem spec.
"""

import os

import numpy as np

# Problem constants
H = 1024  # hidden dim
I = 4096  # intermediate dim
E = 8  # experts
P = 128  # SBUF partitions
IH = I // 2  # intermediate features per core (half of I)

# Tiling constants
TB = 512  # tokens per block (matmul moving free dim)
IS = 1024  # intermediate features resident per weight wave
N_SUPER = IH // IS  # weight waves per segment (= 2)
IT = IS // P  # i-tiles per wave
HO = H // P  # h chunks (contraction tiles)
HH = H // 512  # output column halves for the down projection
NQ = IT // 2  # quarter sub-tiles for the first wave's weight load


def _blocks(Tc):
    """Token blocks, largest first: the first wave's first block consumes
    the just-arriving weights at the slowest rate, and the smallest block
    lands last so each wave's flush is minimal."""
    assert Tc % P == 0 and Tc >= 256
    sizes = []
    rem = Tc
    while rem > 767:
        sizes.append(TB)
        rem -= TB
    if rem > 512:
        sizes.extend([rem - 256, 256])
    elif rem:
        sizes.append(rem)
    sizes.sort(reverse=True)
    blocks = []
    t = 0
    for tb in sizes:
        blocks.append((t, tb))
        t += tb
    return blocks


def build_moe(TCA: int, TCB: int):
    """Per-core program: segments A/B of TCA/TCB tokens, half-I each."""
    import concourse.bass as bass  # noqa: F401
    import concourse.mybir as mybir
    import concourse.tile as tile
    from concourse import bacc

    f32 = mybir.dt.float32
    bf16 = mybir.dt.bfloat16
    Alu = mybir.AluOpType
    Act = mybir.ActivationFunctionType

    nc = bacc.Bacc(
        "TRN2", target_bir_lowering=False, debug=False, num_devices=8
    )

    segs = {}
    for s, Tc in (("a", TCA), ("b", TCB)):
        seg = {
            "Tc": Tc,
            "blocks": _blocks(Tc),
            "NW": Tc // P,
            "xT": nc.dram_tensor(f"x{s}", [H, Tc], bf16, kind="ExternalInput").ap(),
            "wg": nc.dram_tensor(f"wg{s}", [H, IH], bf16, kind="ExternalInput").ap(),
            "wu": nc.dram_tensor(f"wu{s}", [H, IH], bf16, kind="ExternalInput").ap(),
            "wd": nc.dram_tensor(f"wd{s}", [IH, H], bf16, kind="ExternalInput").ap(),
            "wal": nc.dram_tensor(f"wal{s}", [P, Tc // P], f32, kind="ExternalInput").ap(),
            # Wave 0 writes f32, wave 1 writes a separate bf16 buffer on
            # the HWDGE queues (no read-modify-write accumulate anywhere;
            # the host adds the two).
            "out": nc.dram_tensor(f"out{s}", [Tc, H], f32, kind="ExternalOutput").ap(),
            "out2": nc.dram_tensor(f"out2{s}", [Tc, H], bf16, kind="ExternalOutput").ap(),
        }
        seg["xT_r"] = seg["xT"].rearrange("(ho p) t -> p ho t", p=P)
        seg["wg_r"] = seg["wg"].rearrange("(ho p) i -> p ho i", p=P)
        seg["wu_r"] = seg["wu"].rearrange("(ho p) i -> p ho i", p=P)
        seg["wd_r"] = seg["wd"].rearrange("(io p) h -> p io h", p=P)
        segs[s] = seg

    with tile.TileContext(nc) as tc:
        with (
            tc.tile_pool(name="singles", bufs=1) as singles,
            tc.tile_pool(name="xres", bufs=1) as xres,
            tc.tile_pool(name="w0", bufs=1) as w0pool,
            tc.tile_pool(name="weights", bufs=2) as wpool,
            tc.tile_pool(name="hp", bufs=2) as hpool,
            tc.tile_pool(name="ep", bufs=3) as epool,
            tc.tile_pool(name="pgu", bufs=2, space="PSUM") as pgu,
            tc.tile_pool(name="pout", bufs=3, space="PSUM") as pout,
        ):
            # x tiles are SHARED between the two segments (union of their
            # block-size multisets) and reloaded at each wave; the WAR
            # dependencies through the tile framework schedule each
            # reload during the preceding wave.
            from collections import Counter

            need = Counter()
            for seg in segs.values():
                c = Counter(tb for _, tb in seg["blocks"])
                for sz, n in c.items():
                    need[sz] = max(need[sz], n)
            xtiles = {
                sz: [
                    xres.tile([P, HO, sz], bf16, tag=f"xt{sz}_{k}", name=f"xt{sz}_{k}")
                    for k in range(n)
                ]
                for sz, n in need.items()
            }
            for s, seg in segs.items():
                seg["wal_sb"] = singles.tile(
                    [P, seg["NW"]], f32, tag=f"wal{s}", name=f"wal{s}"
                )
                used = Counter()
                seg["x_sb"] = []
                for _, tb in seg["blocks"]:
                    seg["x_sb"].append(xtiles[tb][used[tb]])
                    used[tb] += 1

            # Segment A block 0 loads first on the (fast, otherwise idle)
            # gpsimd queue so the PE can start ~17 us in.
            sa = segs["a"]
            t0_0, tb_0 = sa["blocks"][0]
            nc.gpsimd.dma_start(sa["x_sb"][0], sa["xT_r"][:, :, t0_0 : t0_0 + tb_0])

            # Waves: (segment, super) in execution order.
            waves = [("a", 0), ("b", 0), ("a", 1), ("b", 1)]
            for wi, (s, sup) in enumerate(waves):
                seg = segs[s]
                blocks = seg["blocks"]
                x_sb = seg["x_sb"]
                wal_sb = seg["wal_sb"]
                i0 = sup * IS
                if wi == 0:
                    # First wave's gate/up weights race the PE: quarter
                    # tiles split across the queues in measured-rate order
                    # (SWDGE ~260 GB/s, each HWDGE queue ~60 GB/s).
                    wgq, wuq = [], []
                    for q in range(NQ):
                        wgq.append(
                            w0pool.tile([P, HO, 2 * P], bf16, tag=f"wgq{q}", name=f"wgq{q}")
                        )
                        wuq.append(
                            w0pool.tile([P, HO, 2 * P], bf16, tag=f"wuq{q}", name=f"wuq{q}")
                        )
                    for q, eng_g, eng_u in (
                        (0, nc.sync, nc.scalar),
                        (1, nc.gpsimd, nc.gpsimd),
                        (2, nc.sync, nc.scalar),
                        (3, nc.gpsimd, nc.gpsimd),
                    ):
                        c = i0 + q * 2 * P
                        eng_g.dma_start(wgq[q], seg["wg_r"][:, :, c : c + 2 * P])
                        eng_u.dma_start(wuq[q], seg["wu_r"][:, :, c : c + 2 * P])

                    def wgt(it, wgq=wgq):
                        return wgq[it // 2][:, :, (it % 2) * P : (it % 2 + 1) * P]

                    def wut(it, wuq=wuq):
                        return wuq[it // 2][:, :, (it % 2) * P : (it % 2 + 1) * P]

                    if len(blocks) > 1:
                        nc.gpsimd.dma_start(
                            x_sb[1],
                            seg["xT_r"][:, :, blocks[1][0] : blocks[1][0] + blocks[1][1]],
                        )
                    nc.gpsimd.dma_start(wal_sb, seg["wal"])
                    wd_sb = wpool.tile([P, IT, H], bf16, tag="wd", name="wd")
                    nc.gpsimd.dma_start(
                        wd_sb, seg["wd_r"][:, sup * IT : (sup + 1) * IT, :]
                    )
                    # Remaining x of wave 0, in need order.
                    for bi, (t0, tb) in enumerate(blocks):
                        if bi > 1:
                            nc.gpsimd.dma_start(
                                x_sb[bi], seg["xT_r"][:, :, t0 : t0 + tb]
                            )
                    nc.gpsimd.dma_start(segs["b"]["wal_sb"], segs["b"]["wal"])
                else:
                    # Reload this segment's x into the shared tiles; the
                    # WAR deps on the previous waves' reads schedule these
                    # DMAs during the preceding wave.
                    for bi, (t0, tb) in enumerate(blocks):
                        nc.gpsimd.dma_start(
                            x_sb[bi], seg["xT_r"][:, :, t0 : t0 + tb]
                        )
                    wg_sb = wpool.tile([P, HO, IS], bf16, tag="wg", name="wg_sb")
                    nc.sync.dma_start(wg_sb, seg["wg_r"][:, :, i0 : i0 + IS])
                    wu_sb = wpool.tile([P, HO, IS], bf16, tag="wu", name="wu_sb")
                    nc.scalar.dma_start(wu_sb, seg["wu_r"][:, :, i0 : i0 + IS])

                    def wgt(it, wg_sb=wg_sb):
                        return wg_sb[:, :, it * P : (it + 1) * P]

                    def wut(it, wu_sb=wu_sb):
                        return wu_sb[:, :, it * P : (it + 1) * P]

                    wd_sb = wpool.tile([P, IT, H], bf16, tag="wd", name="wd")
                    nc.sync.dma_start(
                        wd_sb, seg["wd_r"][:, sup * IT : (sup + 1) * IT, :]
                    )

                def down_group(t0, h_sb, grp, seg=seg, sup=sup, wd_sb=wd_sb,
                               wal_sb=wal_sb):
                    # One (token-subtile, output-half) group of the down
                    # projection, back to token-partition layout, scaled by
                    # the combine weight at PSUM eviction. Wave 0 writes
                    # f32; wave 1 writes bf16 via the HWDGE queues so the
                    # SWDGE queue is quiet long before the kernel tail.
                    tsub, hh = divmod(grp, HH)
                    col = t0 // P + tsub
                    r0 = t0 + tsub * P
                    ops = pout.tile([P, 512], f32, tag="o", name="o")
                    for it in range(IT):
                        nc.tensor.matmul(
                            ops,
                            lhsT=h_sb[:, it, tsub * P : (tsub + 1) * P],
                            rhs=wd_sb[:, it, hh * 512 : (hh + 1) * 512],
                            start=(it == 0),
                            stop=(it == IT - 1),
                        )
                    if sup == N_SUPER - 1:
                        oev2 = epool.tile([P, 512], bf16, tag="oev2", name="ov2")
                        nc.vector.tensor_scalar_mul(
                            oev2, ops, wal_sb[:, col : col + 1]
                        )
                        eng = nc.sync if (col + hh) % 2 == 0 else nc.scalar
                        eng.dma_start(
                            seg["out2"][r0 : r0 + P, hh * 512 : (hh + 1) * 512],
                            oev2,
                        )
                    else:
                        oev = epool.tile([P, 512], f32, tag="oev", name="oev")
                        nc.vector.tensor_scalar_mul(
                            oev, ops, wal_sb[:, col : col + 1]
                        )
                        nc.gpsimd.dma_start(
                            seg["out"][r0 : r0 + P, hh * 512 : (hh + 1) * 512],
                            oev,
                        )

                pending = None
                for bi, (t0, tb) in enumerate(blocks):
                    tsn = tb // P
                    # Expert FFN for this (i-chunk, token block):
                    # hT[i, t] = silu(Wg.T x)[i, t] * (Wu.T x)[i, t]
                    h_sb = hpool.tile([P, IT, TB], bf16, tag="h", name="h")[:, :, :tb]
                    dgn = tsn * HH
                    for it in range(IT):
                        gps = pgu.tile([P, TB], f32, tag="g", name="g")[:, :tb]
                        ups = pgu.tile([P, TB], f32, tag="u", name="u")[:, :tb]
                        for ho in range(HO):
                            nc.tensor.matmul(
                                gps,
                                lhsT=wgt(it)[:, ho, :],
                                rhs=x_sb[bi][:, ho, :],
                                start=(ho == 0),
                                stop=(ho == HO - 1),
                            )
                        for ho in range(HO):
                            nc.tensor.matmul(
                                ups,
                                lhsT=wut(it)[:, ho, :],
                                rhs=x_sb[bi][:, ho, :],
                                start=(ho == 0),
                                stop=(ho == HO - 1),
                            )
                        gs = epool.tile([P, TB], f32, tag="gs", name="gs")[:, :tb]
                        nc.scalar.activation(gs, gps, Act.Silu)
                        nc.vector.tensor_tensor(
                            h_sb[:, it, :], gs, ups, op=Alu.mult
                        )
                        if pending is not None:
                            p_t0, p_h, p_dgn, p_dg = pending
                            for grp in range(
                                it * p_dgn // IT, (it + 1) * p_dgn // IT
                            ):
                                p_dg(p_t0, p_h, grp)

                    pending = (t0, h_sb, dgn, down_group)
                if pending is not None:
                    p_t0, p_h, p_dgn, p_dg = pending
                    for grp in range(p_dgn):
                        p_dg(p_t0, p_h, grp)

    nc.compile()
    return nc


def _run_spmd(nc, in_maps, trace):
    from concourse import bass_utils

    if trace:
        try:
            res = bass_utils.run_bass_kernel_spmd(
                nc, in_maps, core_ids=list(range(E)), trace=True
            )
            if res.exec_time_ns is not None:
                print(f"HW exec time: {res.exec_time_ns} ns")
            return res
        except Exception as exc:  # fall back to an untraced run
            print(f"traced run failed ({exc!r}); retrying without trace")
    return bass_utils.run_bass_kernel_spmd(
        nc, in_maps, core_ids=list(range(E)), trace=False
    )


def prepare(hidden_states, gate_proj_w, gate_weights, up_weights, down_weights):
    """Host router + paired dispatch; returns (nc, in_maps, combine_fn)."""
    import ml_dtypes

    bf16 = ml_dtypes.bfloat16
    x = np.ascontiguousarray(hidden_states, dtype=np.float32)
    gpw = np.ascontiguousarray(gate_proj_w, dtype=np.float32)
    T = x.shape[0]

    # Router in fp64: logits -> softmax -> top-2 (stable ties like
    # jax.lax.top_k) -> renormalized combine weights.
    logits = x.astype(np.float64) @ gpw.astype(np.float64).T  # [T, E]
    pr = np.exp(logits - logits.max(axis=1, keepdims=True))
    pr /= pr.sum(axis=1, keepdims=True)
    top2 = np.argsort(-pr, axis=1, kind="stable")[:, :2]
    pv = np.take_along_axis(pr, top2, axis=1)
    wts = (pv / pv.sum(axis=1, keepdims=True)).astype(np.float32)  # [T, 2]

    idx = [np.nonzero((top2 == e).any(axis=1))[0] for e in range(E)]
    cnt = np.array([len(ix) for ix in idx])

    # Pair heaviest with lightest: segment A = 4 heaviest experts,
    # segment B = 4 lightest, pair rank k of A with rank -k of B.
    order = np.argsort(-cnt, kind="stable")
    pairs = [(int(order[k]), int(order[E - 1 - k])) for k in range(E // 2)]

    def pad128(n):
        return max(256, ((n + P - 1) // P) * P)

    TCA = pad128(max(cnt[a] for a, _ in pairs))
    TCB = pad128(max(cnt[b] for _, b in pairs))

    nc = build_moe(TCA, TCB)

    def seg_inputs(e, Tc, half):
        n_e = len(idx[e])
        xTe = np.zeros((H, Tc), dtype=bf16)
        if n_e:
            xTe[:, :n_e] = np.ascontiguousarray(x[idx[e]].T).astype(bf16)
        we = np.zeros((Tc,), dtype=np.float32)
        if n_e:
            we[:n_e] = np.where(
                top2[idx[e], 0] == e, wts[idx[e], 0], wts[idx[e], 1]
            )
        lo, hi = half * IH, (half + 1) * IH
        return {
            "x": xTe,
            "wg": np.ascontiguousarray(gate_weights[e][:, lo:hi]).astype(bf16),
            "wu": np.ascontiguousarray(up_weights[e][:, lo:hi]).astype(bf16),
            "wd": np.ascontiguousarray(down_weights[e][lo:hi, :]).astype(bf16),
            "wal": np.ascontiguousarray(we.reshape(Tc // P, P).T),
        }

    in_maps = []
    core_expert = []  # (expert_a, expert_b) per core
    for a, b in pairs:
        for half in range(2):
            sa = seg_inputs(a, TCA, half)
            sb = seg_inputs(b, TCB, half)
            in_maps.append(
                {
                    "xa": sa["x"], "wga": sa["wg"], "wua": sa["wu"],
                    "wda": sa["wd"], "wala": sa["wal"],
                    "xb": sb["x"], "wgb": sb["wg"], "wub": sb["wu"],
                    "wdb": sb["wd"], "walb": sb["wal"],
                }
            )
            core_expert.append((a, b))

    def combine(results):
        out = np.zeros((T, H), dtype=np.float32)
        for core, (a, b) in enumerate(core_expert):
            r = results[core]
            n_a = len(idx[a])
            if n_a:
                out[idx[a]] += (
                    r["outa"][:n_a] + r["out2a"][:n_a].astype(np.float32)
                )
            n_b = len(idx[b])
            if n_b:
                out[idx[b]] += (
                    r["outb"][:n_b] + r["out2b"][:n_b].astype(np.float32)
                )
        return out

    return nc, in_maps, combine


def kernel(hidden_states, gate_proj_w, gate_weights, up_weights, down_weights):
    trace = os.environ.get("MOE_TRACE", "0") == "1"
    nc, in_maps, combine = prepare(
        hidden_states, gate_proj_w, gate_weights, up_weights, down_weights
    )
    res = _run_spmd(nc, in_maps, trace)
    return combine(res.results)
